# revision 1
# baseline (speedup 1.0000x reference)
"""Multi-head self-attention (RoPE, causal) Bass kernel for 8 TRN2 NeuronCores.

Sharding: tensor-parallel over heads for QKV+attention (2 heads/core),
AllToAll, then token-parallel O-projection (512 tokens/core).

Layouts (per core):
  qT/kT/vT: [128 part = 2 heads x 64 dk, t]  (projection outputs, head-major)
  scoresT:  [128 part = k-tile, q free]      (softmax sum via ones-row matmul)
  v_sb:     [128 part = k-tile tokens, 130]  ([v_h0 | ones | v_h1 | ones])
  aoT:      [128 d, t]  attention output, normalized, pre-O-projection
  y:        [t, o] token-major final output

Causal masking: -1e9 mask matrices accumulated into the score PSUM via an
identity-stationary matmul (keeps masking on the PE, off the DVE/GpSimd).
"""

import numpy as np

B, S, D, H, DK = 2, 2048, 1024, 16, 64
NC = 8
THETA = 10000.0

_COMPILED = {}


def _build():
    import concourse.bass as bass
    import concourse.tile as tile
    from concourse import bacc, mybir

    f32 = mybir.dt.float32
    f32r = mybir.dt.float32r
    MUL = mybir.AluOpType.mult
    ADD = mybir.AluOpType.add
    EXP = mybir.ActivationFunctionType.Exp

    nc = bacc.Bacc(num_devices=NC)

    xt_d = nc.dram_tensor("xt", [B, D, S], f32r, kind="ExternalInput")
    wqt_d = nc.dram_tensor("wqt", [D, 128], f32r, kind="ExternalInput")
    wkt_d = nc.dram_tensor("wkt", [D, 128], f32r, kind="ExternalInput")
    wvt_d = nc.dram_tensor("wvt", [D, 128], f32r, kind="ExternalInput")
    wot_d = nc.dram_tensor("wot", [D, D], f32r, kind="ExternalInput")
    cost_d = nc.dram_tensor("cost", [128, S], f32, kind="ExternalInput")
    sinmt_d = nc.dram_tensor("sinmt", [128, S], f32, kind="ExternalInput")
    masks_d = nc.dram_tensor("masks", [2, 128, 256], f32r, kind="ExternalInput")
    ident_d = nc.dram_tensor("ident", [128, 128], f32, kind="ExternalInput")
    identr_d = nc.dram_tensor("identr", [128, 128], f32r, kind="ExternalInput")
    ones_d = nc.dram_tensor("ones", [128, 16], f32r, kind="ExternalInput")
    y_d = nc.dram_tensor("y", [B, S // NC, D], f32, kind="ExternalOutput")

    SWAP_MASK = [(i ^ 1) for i in range(32)]

    with tile.TileContext(nc) as tc:
        with (
            tc.tile_pool(name="const", bufs=1) as constp,
            tc.tile_pool(name="xtp", bufs=2) as xtp,
            tc.tile_pool(name="qk", bufs=1) as qkp,
            tc.tile_pool(name="vp", bufs=1) as vp,
            tc.tile_pool(name="attn", bufs=3) as attnp,
            tc.tile_pool(name="ao", bufs=1) as aop,
            tc.tile_pool(name="small", bufs=1) as smallp,
            tc.tile_pool(name="rbp", bufs=2) as rbp,
            tc.tile_pool(name="rtmp", bufs=2) as rtmp,
            tc.tile_pool(name="oproj", bufs=1) as op_,
            tc.tile_pool(name="yp", bufs=1) as yp,
            tc.tile_pool(name="ps", bufs=4, space="PSUM") as psp,
            tc.tile_pool(name="dram", bufs=1, space="DRAM") as dramp,
        ):
            # ---- constant tiles (loads emitted as late as their first use allows) ----
            cost = constp.tile([128, S], f32)
            sinmt = constp.tile([128, S], f32)
            masks = constp.tile([128, 2, 256], f32r)
            ident = constp.tile([128, 128], f32)
            identr = constp.tile([128, 128], f32r)
            ones_sb = constp.tile([128, 16], f32r)
            wq_sb = constp.tile([128, 8, 128], f32r)
            wk_sb = constp.tile([128, 8, 128], f32r)
            wv_sb = constp.tile([128, 8, 128], f32r)
            wo_sb = constp.tile([128, 8, D], f32r)

            # critical path: projection weights (sync queue, ahead of xt tiles)
            for dc in range(8):
                dsl = slice(dc * 128, (dc + 1) * 128)
                nc.sync.dma_start(wq_sb[:, dc, :], wqt_d[dsl, :])
                nc.sync.dma_start(wk_sb[:, dc, :], wkt_d[dsl, :])
                nc.sync.dma_start(wv_sb[:, dc, :], wvt_d[dsl, :])

            warm_in = dramp.tile([NC, 64], f32r, name="warm_in")
            warm_out = dramp.tile([NC, 64], f32r, name="warm_out")
            nc.gpsimd.collective_compute(
                "AllToAll",
                mybir.AluOpType.bypass,
                replica_groups=[list(range(NC))],
                ins=[warm_in.opt()],
                outs=[warm_out.opt()],
            )
            a2a_in = [dramp.tile([NC, 128, 256], f32r, name=f"a2ai{i}") for i in range(B)]
            a2a_out = [dramp.tile([NC, 128, 256], f32r, name=f"a2ao{i}") for i in range(B)]
            recip_dram = dramp.tile([B, 8, 512], f32)

            def o_projection(u):
                g = op_.tile([128, 8, 256], f32r, tag="g", name="g")
                for s in range(NC):
                    nc.sync.dma_start(g[:, s, :], a2a_out[u][s])
                y_sb = yp.tile([128, 2, D], f32, tag="y", name="y_sb")
                for ttt in range(2):
                    y_ps = psp.tile([128, 1024], f32, tag="ps", name="y_ps")
                    for os_ in range(2):
                        for dc in range(8):
                            nc.tensor.matmul(
                                y_ps[:, os_ * 512:(os_ + 1) * 512],
                                g[:, dc, ttt * 128:(ttt + 1) * 128],
                                wo_sb[:, dc, os_ * 512:(os_ + 1) * 512],
                                start=(dc == 0), stop=(dc == 7),
                            )
                    nc.vector.tensor_copy(out=y_sb[:, ttt, :], in_=y_ps[:])
                for ttt in range(2):
                    nc.sync.dma_start(y_d[u, ttt * 128:(ttt + 1) * 128, :], y_sb[:, ttt, :])

            for u in range(B):
                # ================= projections + RoPE =================
                qT = qkp.tile([128, S], f32r, tag="qT", name="qT")
                kT = qkp.tile([128, S], f32r, tag="kT", name="kT")
                v_sb = vp.tile([128, 16, 130], f32r, tag="v", name="v_sb")

                for tt in range(4):
                    ts = slice(tt * 512, (tt + 1) * 512)
                    xt_sb = xtp.tile([128, 8, 512], f32r, tag="xt", name="xt_sb")
                    for dc in range(8):
                        nc.sync.dma_start(
                            xt_sb[:, dc, :], xt_d[u, dc * 128:(dc + 1) * 128, ts]
                        )
                    if u == 0 and tt == 0:
                        # non-critical consts: emitted after the first xt tile
                        nc.gpsimd.dma_start(ident[:], ident_d[:])
                        nc.gpsimd.dma_start(ones_sb[:], ones_d[:])
                        nc.gpsimd.dma_start(cost[:], cost_d[:])
                        nc.gpsimd.dma_start(sinmt[:], sinmt_d[:])
                        nc.gpsimd.dma_start(masks[:, 0, :], masks_d[0])
                        nc.gpsimd.dma_start(masks[:, 1, :], masks_d[1])
                        nc.gpsimd.dma_start(identr[:], identr_d[:])
                    qk_ps = psp.tile([128, 1024], f32, tag="ps", name="qk_ps")
                    v_ps = psp.tile([128, 1024], f32, tag="ps", name="v_ps")
                    for dc in range(8):
                        st = dc == 0
                        sp = dc == 7
                        rhs = xt_sb[:, dc, :]
                        nc.tensor.matmul(qk_ps[:, 0:512], wq_sb[:, dc, :], rhs, start=st, stop=sp)
                        nc.tensor.matmul(qk_ps[:, 512:1024], wk_sb[:, dc, :], rhs, start=st, stop=sp)
                        nc.tensor.matmul(v_ps[:, 0:512], wv_sb[:, dc, :], rhs, start=st, stop=sp)

                    # RoPE: dst = q*cos + pairswap(q)*sinm
                    for src, dst in ((qk_ps[:, 0:512], qT), (qk_ps[:, 512:1024], kT)):
                        qs = rtmp.tile([128, 512], f32, tag="qs", name="qs")
                        t2 = rtmp.tile([128, 512], f32, tag="t2", name="t2")
                        nc.vector.stream_shuffle(qs[:], src, SWAP_MASK)
                        nc.vector.tensor_tensor(out=dst[:, ts], in0=src, in1=cost[:, ts], op=MUL)
                        nc.vector.tensor_tensor(out=t2[:], in0=qs[:], in1=sinmt[:, ts], op=MUL)
                        nc.vector.tensor_tensor(out=dst[:, ts], in0=dst[:, ts], in1=t2[:], op=ADD)

                    # v -> token-major via PE transpose; ones columns appended
                    vtmp = rtmp.tile([128, 512], f32, tag="vtmp", name="vtmp")
                    nc.vector.tensor_copy(out=vtmp[:], in_=v_ps[:, 0:512])
                    for s4 in range(4):
                        kt = tt * 4 + s4
                        tr = v_ps[:, 512 + s4 * 128: 512 + (s4 + 1) * 128]
                        nc.tensor.transpose(tr, vtmp[:, s4 * 128:(s4 + 1) * 128], ident[:])
                        dst = v_sb[:, kt, :].rearrange("p (u c) -> p u c", u=2)[:, :, 0:64]
                        src = tr.rearrange("p (u c) -> p u c", u=2)
                        nc.vector.tensor_copy(out=dst, in_=src)
                    nc.vector.tensor_copy(out=v_sb[:, tt * 4:(tt + 1) * 4, 64:65],
                                          in_=ones_sb[:, tt * 4:(tt + 1) * 4])
                    nc.vector.tensor_copy(out=v_sb[:, tt * 4:(tt + 1) * 4, 129:130],
                                          in_=ones_sb[:, tt * 4:(tt + 1) * 4])

                # ================= attention (normalize+ship per q-tile) =================
                aoT = aop.tile([128, S], f32r, tag="aoT", name="aoT")
                recip = smallp.tile([1, 8, 512], f32, tag="recip", name="recip")
                for qi in range(4):
                    qsl = slice(qi * 512, (qi + 1) * 512)
                    outT = psp.tile([128, 1024], f32, tag="ps", name="outT")
                    n_kt = 4 * qi + 4
                    for kt in range(n_kt):
                        ksl = slice(kt * 128, (kt + 1) * 128)
                        diag_pos = kt - 4 * qi
                        sc = psp.tile([128, 1024], f32, tag="ps", name="sc")
                        at = attnp.tile([128, 1024], f32r, tag="at", name="at")
                        if diag_pos < 2:
                            for h in (0, 1):
                                hp = slice(h * 64, (h + 1) * 64)
                                nc.tensor.matmul(
                                    sc[:, h * 512:(h + 1) * 512],
                                    kT[hp, ksl],
                                    qT[hp, qsl],
                                    start=True, stop=(diag_pos < 0),
                                    skip_group_check=True,
                                )
                            if diag_pos >= 0:
                                # causal mask: accumulate -1e9 pattern via identity matmul
                                for h in (0, 1):
                                    nc.tensor.matmul(
                                        sc[:, h * 512: h * 512 + 256],
                                        identr[:],
                                        masks[:, diag_pos, :],
                                        start=False, stop=True,
                                        skip_group_check=True,
                                    )
                            nc.scalar.activation(out=at[:], in_=sc[:], func=EXP, scale=0.125)
                            for h in (0, 1):
                                nc.tensor.matmul(
                                    outT[0:65, h * 512:(h + 1) * 512],
                                    v_sb[:, kt, :].rearrange("p (u c) -> p u c", u=2)[:, h, :],
                                    at[:, h * 512:(h + 1) * 512],
                                    start=(kt == 0), stop=(kt == n_kt - 1),
                                    skip_group_check=True,
                                )
                        else:
                            # kt2/kt3 of the diagonal: only q columns 256:512
                            for h in (0, 1):
                                hp = slice(h * 64, (h + 1) * 64)
                                nc.tensor.matmul(
                                    sc[:, h * 512 + 256: h * 512 + 512],
                                    kT[hp, ksl],
                                    qT[hp, qsl][:, 256:512],
                                    start=True, stop=False,
                                    skip_group_check=True,
                                )
                                nc.tensor.matmul(
                                    sc[:, h * 512 + 256: h * 512 + 512],
                                    identr[:],
                                    masks[:, diag_pos - 2, :],
                                    start=False, stop=True,
                                    skip_group_check=True,
                                )
                            scs = sc.rearrange("p (h q) -> p h q", h=2)[:, :, 256:512]
                            ats = at.rearrange("p (h q) -> p h q", h=2)[:, :, 256:512]
                            nc.scalar.activation(out=ats, in_=scs, func=EXP, scale=0.125)
                            for h in (0, 1):
                                nc.tensor.matmul(
                                    outT[0:65, h * 512 + 256: h * 512 + 512],
                                    v_sb[:, kt, :].rearrange("p (u c) -> p u c", u=2)[:, h, :],
                                    at[:, h * 512 + 256: h * 512 + 512],
                                    start=False, stop=(kt == n_kt - 1),
                                    skip_group_check=True,
                                )
                    # unload outT; normalize + ship this q-tile immediately
                    dent = smallp.tile([1, 512], f32, tag="dent", name="dent", bufs=2)
                    for h in (0, 1):
                        nc.vector.tensor_copy(
                            out=aoT[h * 64:(h + 1) * 64, qsl],
                            in_=outT[0:64, h * 512:(h + 1) * 512],
                        )
                        nc.vector.tensor_copy(
                            out=dent[0:1, :], in_=outT[64:65, h * 512:(h + 1) * 512]
                        )
                        nc.vector.reciprocal_approx_fast(
                            out=recip[0:1, h * 4 + qi, :], in_=dent[0:1, :]
                        )
                        nc.sync.dma_start(
                            recip_dram[u, h * 4 + qi: h * 4 + qi + 1, :],
                            recip[0:1, h * 4 + qi, :],
                        )
                    rb = rbp.tile([128, 512], f32, tag="rb", name="rb")
                    for h in (0, 1):
                        nc.gpsimd.dma_start(
                            rb[h * 64:(h + 1) * 64, :],
                            recip_dram[u, h * 4 + qi: h * 4 + qi + 1, :].to_broadcast([64, 512]),
                        )
                    nc.vector.tensor_tensor(out=aoT[:, qsl], in0=aoT[:, qsl], in1=rb[:], op=MUL)
                    for s in (2 * qi, 2 * qi + 1):
                        nc.sync.dma_start(a2a_in[u][s], aoT[:, s * 256:(s + 1) * 256])

                if u == 0:
                    # O-projection weights: off the startup critical path
                    for dc in range(8):
                        nc.gpsimd.dma_start(wo_sb[:, dc, :], wot_d[dc * 128:(dc + 1) * 128, :])

                if u > 0:
                    # unit u-1's O-projection must be emitted BEFORE unit u's
                    # collective: reads of a2a_out[u-1] otherwise wait on this
                    # collective too (collective completions share one
                    # cumulative semaphore).
                    o_projection(u - 1)
                nc.gpsimd.collective_compute(
                    "AllToAll",
                    mybir.AluOpType.bypass,
                    replica_groups=[list(range(NC))],
                    ins=[a2a_in[u].opt()],
                    outs=[a2a_out[u].opt()],
                )

            o_projection(B - 1)

    nc.compile()
    return nc


def _host_inputs(x, wq, wk, wv, wo):
    xt = np.ascontiguousarray(x.transpose(0, 2, 1))
    wot = np.ascontiguousarray(wo.T)

    p = np.arange(128)
    invf = THETA ** (-2.0 * ((p % 64) // 2) / 64.0)
    ang = invf[:, None] * np.arange(S)[None, :]
    cost = np.cos(ang).astype(np.float32)
    sinmt = (np.sin(ang) * np.where(p % 2 == 0, -1.0, 1.0)[:, None]).astype(np.float32)

    i = np.arange(128)[:, None]
    j = np.arange(256)[None, :]
    # additive causal masks: 0 where allowed (j >= i + off), -1e9 where masked
    masks = np.stack([
        np.where(j >= i + 0, 0.0, -1e9).astype(np.float32),
        np.where(j >= i + 128, 0.0, -1e9).astype(np.float32),
    ])
    ident = np.eye(128, dtype=np.float32)

    in_maps = []
    for c in range(NC):
        sl = slice(c * 128, (c + 1) * 128)
        in_maps.append({
            "xt": xt,
            "wqt": np.ascontiguousarray(wq[sl, :].T),
            "wkt": np.ascontiguousarray(wk[sl, :].T),
            "wvt": np.ascontiguousarray(wv[sl, :].T),
            "wot": wot,
            "cost": cost,
            "sinmt": sinmt,
            "masks": masks,
            "ident": ident,
            "identr": ident,
            "ones": np.ones((128, 16), np.float32),
        })
    return in_maps


def kernel(x, wq, wk, wv, wo, _trace=False):
    from concourse.bass_utils import run_bass_kernel_spmd

    if "nc" not in _COMPILED:
        _COMPILED["nc"] = _build()
    nc = _COMPILED["nc"]

    in_maps = _host_inputs(
        np.asarray(x, np.float32), np.asarray(wq, np.float32),
        np.asarray(wk, np.float32), np.asarray(wv, np.float32),
        np.asarray(wo, np.float32),
    )
    res = run_bass_kernel_spmd(nc, in_maps, core_ids=list(range(NC)), trace=_trace)
    _COMPILED["last_result"] = res

    y = np.zeros((B, S, D), np.float32)
    for c in range(NC):
        yc = res.results[c]["y"]
        y[:, c * 256:(c + 1) * 256, :] = yc
    return y

